# revision 5
# baseline (speedup 1.0000x reference)
"""Trainium2 Bass kernel for sigmoid-gated multi-head attention.

Reference computation (B=4, F=256, H=8, S=1024):
    qx  = q_input^T          (b, s, f)
    q   = qx @ Wq  -> (b, s, f, h)   [col fi*H + hi]
    k,v = kvx @ Wk / Wv
    attn = sigmoid(sqrt(F) * q.k)    per head
    wv   = attn @ v
    out  = relu(concat_heads(wv) @ Wz)   returned as (b, f, s)

Sharding: 8 cores = 4 batches x 2 query-sequence halves. Each core
computes the full pipeline (all 8 heads) for its (batch, s-half) slice,
including the final ReLU, so per-core outputs are disjoint slices of the
final output and no cross-core reduction is needed.  The cost is that
K/V projections are computed by both cores of a batch pair (~14% extra
matmul work vs. the ideal), in exchange for zero collectives.

All on-chip compute keeps the "transposed" layout (feature, sequence),
which matches the DRAM layout of q_input/kv_input and the required
output layout, so no transposes are ever needed:
    QT_h (f, i)  = Wq_h^T @ q_in       KT_h (f, j) = Wk_h^T @ kv_in
    V_h  (j, f)  = kv_in^T @ Wv_h
    attnT_h (j, i) = sigmoid(16 * KT_h^T_slice . QT_h)
    wvT_h (f, i) = V_h^T_slice @ attnT_h
    outT (fo, i) += Wz_h^T @ wvT_h     -> relu -> output slice

All matmuls run as fp32r (full PE rate at N>=256, ~1e-3 rel err).
Inputs are host-packed partition-major so every DRAM->SBUF transfer is
one large contiguous-per-partition DMA (stripes across all 16 SDMA
engines): one DMA for qin, one for kvin, one per head for all four
weight matrices, one for the output.
"""

import os
import sys

sys.path.insert(0, "/opt/trn_rl_repo")

import numpy as np

B, F, H, S = 4, 256, 8, 1024
HALF = S // 2  # query columns per core
NCORES = 8
P = 128  # partitions

_cache = {}


def _build():
    import concourse.mybir as mybir
    import concourse.tile as tile
    from concourse import bacc

    dt = mybir.dt
    f32 = dt.float32
    mm_fast = os.environ.get("ATTN_MM_DTYPE", "fp32r") == "fp32r"
    # dtype for all matmul-input tensors (DRAM + SBUF). walrus requires
    # fp32r matmul operands to be *produced* as fp32r, so the tiles and
    # the copies/activations that fill them carry this dtype directly.
    mdt = dt.float32r if mm_fast else dt.float32
    AF = mybir.ActivationFunctionType

    nc = bacc.Bacc(None, target_bir_lowering=False)

    # all partition-major: [P, ...] with per-partition lines contiguous
    qin_d = nc.dram_tensor("qin", [P, 2, HALF], mdt, kind="ExternalInput")
    kvin_d = nc.dram_tensor("kvin", [P, 2, S], mdt, kind="ExternalInput")
    # per head: [wq|wk|wv|wz][f_in chunk][f_out]
    w_d = nc.dram_tensor("w", [H, P, 4, 2, F], mdt, kind="ExternalInput")
    out_d = nc.dram_tensor("out", [P, 2, HALF], f32, kind="ExternalOutput")

    with tile.TileContext(nc) as tc:
        with (
            tc.tile_pool(name="io", bufs=1) as io_pool,
            tc.tile_pool(name="wts", bufs=2) as w_pool,
            tc.tile_pool(name="qkv", bufs=2) as qkv_pool,
            tc.tile_pool(name="attn", bufs=2) as attn_pool,
            tc.tile_pool(name="ps", bufs=6, space="PSUM") as ps_pool,
            tc.tile_pool(name="ops", bufs=1, space="PSUM") as out_ps_pool,
        ):
            qin = io_pool.tile([P, 2, HALF], mdt, tag="qin")
            kvin = io_pool.tile([P, 2, S], mdt, tag="kvin")
            # qin on the SP HWDGE ring, kvin on the ACT ring: both input
            # transfers start immediately and run in parallel.
            nc.sync.dma_start(qin[:], qin_d[:])
            nc.scalar.dma_start(kvin[:], kvin_d[:])

            # persistent accumulator for the output projection: 2 banks
            out_ps = out_ps_pool.tile([P, 2, HALF], f32, tag="out_ps")

            for h in range(H):
                # one DMA per head for all four weight matrices
                w = w_pool.tile([P, 4, 2, F], mdt, tag="w")
                nc.sync.dma_start(w[:], w_d[h])
                wq = w[:, 0]
                wk = w[:, 1]
                wv = w[:, 2]
                wz = w[:, 3]

                # QT_h (f 2x128, i 512) = Wq_h^T @ qin
                qt = qkv_pool.tile([P, 2, HALF], mdt, tag="qt")
                for t in range(2):
                    ps = ps_pool.tile([P, HALF], f32, tag="ps")
                    for c in range(2):
                        nc.tensor.matmul(
                            ps[:],
                            wq[:, c, P * t : P * (t + 1)],
                            qin[:, c, :],
                            start=(c == 0),
                            stop=(c == 1),
                        )
                    nc.vector.tensor_copy(qt[:, t, :], ps[:])

                # KT_h (f 2x128, j 1024) = Wk_h^T @ kvin
                kt = qkv_pool.tile([P, 2, S], mdt, tag="kt")
                for t in range(2):
                    for n in range(2):
                        ps = ps_pool.tile([P, HALF], f32, tag="ps")
                        for c in range(2):
                            nc.tensor.matmul(
                                ps[:],
                                wk[:, c, P * t : P * (t + 1)],
                                kvin[:, c, HALF * n : HALF * (n + 1)],
                                start=(c == 0),
                                stop=(c == 1),
                            )
                        nc.vector.tensor_copy(kt[:, t, HALF * n : HALF * (n + 1)], ps[:])

                # V_h (j 8x128, f 256) = kvin^T @ Wv_h
                v = qkv_pool.tile([P, H, F], mdt, tag="v")
                for jb in range(8):
                    ps = ps_pool.tile([P, HALF], f32, tag="ps")
                    for c in range(2):
                        nc.tensor.matmul(
                            ps[:, :F],
                            kvin[:, c, P * jb : P * (jb + 1)],
                            wv[:, c, :],
                            start=(c == 0),
                            stop=(c == 1),
                        )
                    nc.vector.tensor_copy(v[:, jb, :], ps[:, :F])

                # attnT_h (j 8x128, i 512) = sigmoid(16 * KT_slice^T @ QT)
                atn = attn_pool.tile([P, 8, HALF], mdt, tag="atn")
                for jb in range(8):
                    ps = ps_pool.tile([P, HALF], f32, tag="ps")
                    for c in range(2):
                        nc.tensor.matmul(
                            ps[:],
                            kt[:, c, P * jb : P * (jb + 1)],
                            qt[:, c, :],
                            start=(c == 0),
                            stop=(c == 1),
                        )
                    nc.scalar.activation(atn[:, jb, :], ps[:], AF.Sigmoid, scale=16.0)

                # wvT_h (f 2x128, i 512) = V_slice^T @ attnT
                wvt = qkv_pool.tile([P, 2, HALF], mdt, tag="wvt")
                for t in range(2):
                    ps = ps_pool.tile([P, HALF], f32, tag="ps")
                    for jb in range(8):
                        nc.tensor.matmul(
                            ps[:],
                            v[:, jb, P * t : P * (t + 1)],
                            atn[:, jb, :],
                            start=(jb == 0),
                            stop=(jb == 7),
                        )
                    nc.vector.tensor_copy(wvt[:, t, :], ps[:])

                # output projection accumulation: outT (fo 2x128, i 512)
                for t in range(2):
                    for c in range(2):
                        nc.tensor.matmul(
                            out_ps[:, t, :],
                            wz[:, c, P * t : P * (t + 1)],
                            wvt[:, c, :],
                            start=(h == 0 and c == 0),
                            stop=(h == H - 1 and c == 1),
                        )

            out_sb = io_pool.tile([P, 2, HALF], f32, tag="out_sb")
            for t in range(2):
                nc.scalar.activation(out_sb[:, t, :], out_ps[:, t, :], AF.Relu)
            nc.scalar.dma_start(out_d[:], out_sb[:])

    nc.compile()
    return nc


def _get_nc():
    key = os.environ.get("ATTN_MM_DTYPE", "fp32r")
    if key not in _cache:
        _cache[key] = _build()
    return _cache[key]


def _make_in_maps(inputs):
    q_input = np.asarray(inputs["q_input"], dtype=np.float32)
    kv_input = np.asarray(inputs["kv_input"], dtype=np.float32)

    # Wq/Wk/Wv [f_in, fo*H] (col fi*H+hi) -> [h, f_in(chunk c, p), fo]
    def cols_by_head(W):
        return np.asarray(W, dtype=np.float32).reshape(2, P, F, H).transpose(3, 0, 1, 2)

    # Wz [f*H, fo] (row fi*H+hi) -> [h, f(chunk c, p), fo]
    WzR = (
        np.asarray(inputs["Wz"], dtype=np.float32)
        .reshape(2, P, H, F)
        .transpose(2, 0, 1, 3)
    )
    # stack to [H, 4, 2, P, F] then to partition-major [H, P, 4, 2, F]
    WALL = np.stack(
        [
            cols_by_head(inputs["Wq"]),
            cols_by_head(inputs["Wk"]),
            cols_by_head(inputs["Wv"]),
            WzR,
        ],
        axis=1,
    )  # [H, 4, 2, P, F]
    WALL = np.ascontiguousarray(WALL.transpose(0, 3, 1, 2, 4))  # [H, P, 4, 2, F]

    in_maps = []
    for c in range(NCORES):
        b, half = divmod(c, 2)
        # q_input[b] (256, 1024) -> [p, chunk, i-half]
        qb = q_input[b].reshape(2, P, S)
        qin = np.ascontiguousarray(
            qb[:, :, half * HALF : (half + 1) * HALF].transpose(1, 0, 2)
        )
        kvin = np.ascontiguousarray(kv_input[b].reshape(2, P, S).transpose(1, 0, 2))
        in_maps.append({"qin": qin, "kvin": kvin, "w": WALL})
    return in_maps


def kernel(q_input, kv_input, Wq, Wk, Wv, Wz, **kw):
    from concourse.bass_utils import run_bass_kernel_spmd

    nc = _get_nc()
    in_maps = _make_in_maps(
        {
            "q_input": q_input,
            "kv_input": kv_input,
            "Wq": Wq,
            "Wk": Wk,
            "Wv": Wv,
            "Wz": Wz,
        }
    )

    res = run_bass_kernel_spmd(nc, in_maps, core_ids=list(range(NCORES)))

    out = np.empty((B, F, S), dtype=np.float32)
    for c in range(NCORES):
        b, half = divmod(c, 2)
        # out dram [p, chunk, i] -> out[b, chunk*128+p, half*512+i]
        o = res.results[c]["out"]  # (P, 2, HALF)
        out[b, :, half * HALF : (half + 1) * HALF] = o.transpose(1, 0, 2).reshape(
            F, HALF
        )
    return out


# revision 8
# speedup vs baseline: 1.0100x; 1.0100x over previous
"""Trainium2 Bass kernel for sigmoid-gated multi-head attention.

Reference computation (B=4, F=256, H=8, S=1024):
    qx  = q_input^T          (b, s, f)
    q   = qx @ Wq  -> (b, s, f, h)   [col fi*H + hi]
    k,v = kvx @ Wk / Wv
    attn = sigmoid(sqrt(F) * q.k)    per head
    wv   = attn @ v
    out  = relu(concat_heads(wv) @ Wz)   returned as (b, f, s)

Sharding: 8 cores = 4 batches x 2 query-sequence halves. Each core
computes the full pipeline (all 8 heads) for its (batch, s-half) slice,
including the final ReLU, so per-core outputs are disjoint slices of the
final output and no cross-core reduction is needed.  The cost is that
K/V projections are computed by both cores of a batch pair (~14% extra
matmul work vs. the ideal), in exchange for zero collectives.

All on-chip compute keeps the "transposed" layout (feature, sequence),
which matches the DRAM layout of q_input/kv_input and the required
output layout, so no transposes are ever needed:
    QT_h (f, i)  = Wq_h^T @ q_in       KT_h (f, j) = Wk_h^T @ kv_in
    V_h  (j, f)  = kv_in^T @ Wv_h
    attnT_h (j, i) = sigmoid(16 * KT_h^T_slice . QT_h)
    wvT_h (f, i) = V_h^T_slice @ attnT_h
    outT (fo, i) += Wz_h^T @ wvT_h     -> relu -> output slice

All matmuls run as fp32r (full PE rate at N>=256, ~1e-3 rel err).
Inputs are host-packed partition-major so every DRAM->SBUF transfer is
one large contiguous-per-partition DMA (stripes across all 16 SDMA
engines): one DMA for qin, one for kvin, one per head for all four
weight matrices, one for the output.
"""

import os
import sys

sys.path.insert(0, "/opt/trn_rl_repo")

import numpy as np

B, F, H, S = 4, 256, 8, 1024
HALF = S // 2  # query columns per core
NCORES = 8
P = 128  # partitions

_cache = {}


def _build():
    import concourse.mybir as mybir
    import concourse.tile as tile
    from concourse import bacc

    dt = mybir.dt
    f32 = dt.float32
    mm_fast = os.environ.get("ATTN_MM_DTYPE", "fp32r") == "fp32r"
    # dtype for all matmul-input tensors (DRAM + SBUF). walrus requires
    # fp32r matmul operands to be *produced* as fp32r, so the tiles and
    # the copies/activations that fill them carry this dtype directly.
    mdt = dt.float32r if mm_fast else dt.float32
    AF = mybir.ActivationFunctionType

    nc = bacc.Bacc(None, target_bir_lowering=False)

    # all partition-major: [P, ...] with per-partition lines contiguous
    qin_d = nc.dram_tensor("qin", [P, 2, HALF], mdt, kind="ExternalInput")
    kvin_d = nc.dram_tensor("kvin", [P, 2, S], mdt, kind="ExternalInput")
    # per head: [wq|wk|wv|wz][f_in chunk][f_out]
    w_d = nc.dram_tensor("w", [H, P, 4, 2, F], mdt, kind="ExternalInput")
    out_d = nc.dram_tensor("out", [P, 2, HALF], f32, kind="ExternalOutput")

    with tile.TileContext(nc) as tc:
        with (
            tc.tile_pool(name="io", bufs=1) as io_pool,
            tc.tile_pool(name="wts", bufs=2) as w_pool,
            tc.tile_pool(name="qkv", bufs=2) as qkv_pool,
            tc.tile_pool(name="attn", bufs=2) as attn_pool,
            tc.tile_pool(name="ps", bufs=6, space="PSUM") as ps_pool,
            tc.tile_pool(name="ops", bufs=1, space="PSUM") as out_ps_pool,
        ):
            # PE pre-warm: dummy matmuls on a zeroed tile keep the PE busy
            # through its HAM activity window while the first input DMAs
            # are in flight, so the real matmuls start at 2.4 GHz instead
            # of paying the ~3.4us half-clock ramp.
            nwarm = int(os.environ.get("ATTN_NWARM", "28"))
            if nwarm:
                warm_f = io_pool.tile([P, HALF], f32, tag="warm_f")
                nc.vector.memset(warm_f[:], 0.0)
                warm = io_pool.tile([P, HALF], mdt, tag="warm")
                nc.vector.tensor_copy(warm[:], warm_f[:])
                wps = ps_pool.tile([P, HALF], f32, tag="ps")
                for _ in range(nwarm):
                    nc.tensor.matmul(
                        wps[:], warm[:, :P], warm[:], start=True, stop=True
                    )

            qin = io_pool.tile([P, 2, HALF], mdt, tag="qin")
            kvin = io_pool.tile([P, 2, S], mdt, tag="kvin")
            # qin on the SP HWDGE ring, kvin on the ACT ring: both input
            # transfers start immediately and run in parallel.
            nc.sync.dma_start(qin[:], qin_d[:])
            nc.scalar.dma_start(kvin[:], kvin_d[:])

            # persistent accumulator for the output projection: 2 banks
            out_ps = out_ps_pool.tile([P, 2, HALF], f32, tag="out_ps")

            for h in range(H):
                # wq split out of the per-head weight block so the very
                # first projection matmuls only wait on qin+wq (0.75 MB),
                # not the whole weight block.
                wq = w_pool.tile([P, 2, F], mdt, tag="wq")
                nc.sync.dma_start(wq[:], w_d[h, :, 0])
                wrest = w_pool.tile([P, 3, 2, F], mdt, tag="wrest")
                nc.scalar.dma_start(wrest[:], w_d[h, :, 1:4])
                wk = wrest[:, 0]
                wv = wrest[:, 1]
                wz = wrest[:, 2]

                # QT_h (f 2x128, i 512) = Wq_h^T @ qin
                qt = qkv_pool.tile([P, 2, HALF], mdt, tag="qt")
                for t in range(2):
                    ps = ps_pool.tile([P, HALF], f32, tag="ps")
                    for c in range(2):
                        nc.tensor.matmul(
                            ps[:],
                            wq[:, c, P * t : P * (t + 1)],
                            qin[:, c, :],
                            start=(c == 0),
                            stop=(c == 1),
                        )
                    nc.vector.tensor_copy(qt[:, t, :], ps[:])

                # KT_h (f 2x128, j 1024) = Wk_h^T @ kvin
                kt = qkv_pool.tile([P, 2, S], mdt, tag="kt")
                for t in range(2):
                    for n in range(2):
                        ps = ps_pool.tile([P, HALF], f32, tag="ps")
                        for c in range(2):
                            nc.tensor.matmul(
                                ps[:],
                                wk[:, c, P * t : P * (t + 1)],
                                kvin[:, c, HALF * n : HALF * (n + 1)],
                                start=(c == 0),
                                stop=(c == 1),
                            )
                        nc.vector.tensor_copy(kt[:, t, HALF * n : HALF * (n + 1)], ps[:])

                # V_h (j 8x128, f 256) = kvin^T @ Wv_h
                v = qkv_pool.tile([P, H, F], mdt, tag="v")
                for jb in range(8):
                    ps = ps_pool.tile([P, HALF], f32, tag="ps")
                    for c in range(2):
                        nc.tensor.matmul(
                            ps[:, :F],
                            kvin[:, c, P * jb : P * (jb + 1)],
                            wv[:, c, :],
                            start=(c == 0),
                            stop=(c == 1),
                        )
                    nc.vector.tensor_copy(v[:, jb, :], ps[:, :F])

                # attnT_h (j 8x128, i 512) = sigmoid(16 * KT_slice^T @ QT)
                atn = attn_pool.tile([P, 8, HALF], mdt, tag="atn")
                for jb in range(8):
                    ps = ps_pool.tile([P, HALF], f32, tag="ps")
                    for c in range(2):
                        nc.tensor.matmul(
                            ps[:],
                            kt[:, c, P * jb : P * (jb + 1)],
                            qt[:, c, :],
                            start=(c == 0),
                            stop=(c == 1),
                        )
                    nc.scalar.activation(atn[:, jb, :], ps[:], AF.Sigmoid, scale=16.0)

                # wvT_h (f 2x128, i 512) = V_slice^T @ attnT
                wvt = qkv_pool.tile([P, 2, HALF], mdt, tag="wvt")
                for t in range(2):
                    ps = ps_pool.tile([P, HALF], f32, tag="ps")
                    for jb in range(8):
                        nc.tensor.matmul(
                            ps[:],
                            v[:, jb, P * t : P * (t + 1)],
                            atn[:, jb, :],
                            start=(jb == 0),
                            stop=(jb == 7),
                        )
                    nc.vector.tensor_copy(wvt[:, t, :], ps[:])

                # output projection accumulation: outT (fo 2x128, i 512)
                for t in range(2):
                    for c in range(2):
                        nc.tensor.matmul(
                            out_ps[:, t, :],
                            wz[:, c, P * t : P * (t + 1)],
                            wvt[:, c, :],
                            start=(h == 0 and c == 0),
                            stop=(h == H - 1 and c == 1),
                        )

            out_sb = io_pool.tile([P, 2, HALF], f32, tag="out_sb")
            for t in range(2):
                nc.scalar.activation(out_sb[:, t, :], out_ps[:, t, :], AF.Relu)
            nc.scalar.dma_start(out_d[:], out_sb[:])

    nc.compile()
    return nc


def _get_nc():
    key = os.environ.get("ATTN_MM_DTYPE", "fp32r")
    if key not in _cache:
        _cache[key] = _build()
    return _cache[key]


def _make_in_maps(inputs):
    q_input = np.asarray(inputs["q_input"], dtype=np.float32)
    kv_input = np.asarray(inputs["kv_input"], dtype=np.float32)

    # Wq/Wk/Wv [f_in, fo*H] (col fi*H+hi) -> [h, f_in(chunk c, p), fo]
    def cols_by_head(W):
        return np.asarray(W, dtype=np.float32).reshape(2, P, F, H).transpose(3, 0, 1, 2)

    # Wz [f*H, fo] (row fi*H+hi) -> [h, f(chunk c, p), fo]
    WzR = (
        np.asarray(inputs["Wz"], dtype=np.float32)
        .reshape(2, P, H, F)
        .transpose(2, 0, 1, 3)
    )
    # stack to [H, 4, 2, P, F] then to partition-major [H, P, 4, 2, F]
    WALL = np.stack(
        [
            cols_by_head(inputs["Wq"]),
            cols_by_head(inputs["Wk"]),
            cols_by_head(inputs["Wv"]),
            WzR,
        ],
        axis=1,
    )  # [H, 4, 2, P, F]
    WALL = np.ascontiguousarray(WALL.transpose(0, 3, 1, 2, 4))  # [H, P, 4, 2, F]

    in_maps = []
    for c in range(NCORES):
        b, half = divmod(c, 2)
        # q_input[b] (256, 1024) -> [p, chunk, i-half]
        qb = q_input[b].reshape(2, P, S)
        qin = np.ascontiguousarray(
            qb[:, :, half * HALF : (half + 1) * HALF].transpose(1, 0, 2)
        )
        kvin = np.ascontiguousarray(kv_input[b].reshape(2, P, S).transpose(1, 0, 2))
        in_maps.append({"qin": qin, "kvin": kvin, "w": WALL})
    return in_maps


def kernel(q_input, kv_input, Wq, Wk, Wv, Wz, **kw):
    from concourse.bass_utils import run_bass_kernel_spmd

    nc = _get_nc()
    in_maps = _make_in_maps(
        {
            "q_input": q_input,
            "kv_input": kv_input,
            "Wq": Wq,
            "Wk": Wk,
            "Wv": Wv,
            "Wz": Wz,
        }
    )

    res = run_bass_kernel_spmd(nc, in_maps, core_ids=list(range(NCORES)))

    out = np.empty((B, F, S), dtype=np.float32)
    for c in range(NCORES):
        b, half = divmod(c, 2)
        # out dram [p, chunk, i] -> out[b, chunk*128+p, half*512+i]
        o = res.results[c]["out"]  # (P, 2, HALF)
        out[b, :, half * HALF : (half + 1) * HALF] = o.transpose(1, 0, 2).reshape(
            F, HALF
        )
    return out
